# revision 13
# baseline (speedup 1.0000x reference)
"""Contextual patch attention (B=4, Cin=64, H=W=128) on 8 trn2 NeuronCores.

Sharding: core = img*2 + half. Each core handles one image's 512 query
patches (half of 1024) against all 16384 key/value patches of that image.
Host (numpy) does the cheap 1x1 convs + unfold layout prep and the final
fold / output conv / residual; the device does the ~210 GFLOP attention.
"""

import numpy as np
from contextlib import ExitStack
from numpy.lib.stride_tricks import as_strided

KS = 7
PAD = 3
SQ = 4          # query stride
SCALE = 10.0
B, CIN, H, W = 4, 64, 128, 128
CIT = 16
NQ = 32         # query grid 32x32
L2 = 128 * 128
D = CIT * KS * KS          # 784
NG = 7                     # contraction groups of 128 (pad 49 offsets -> 56)
QPT = 512                  # queries per core
NKT = 32                   # kv tiles of 512

_cache = {}


def _build_bass(repeat=1):
    import concourse.bass as bass
    import concourse.mybir as mybir
    import concourse.tile as tile
    from concourse import bacc

    fp16 = mybir.dt.float16
    fp32 = mybir.dt.float32

    nc = bacc.Bacc("TRN2", target_bir_lowering=False, debug=False, num_devices=8)
    qt = nc.dram_tensor("qt", [128, NG, QPT], fp16, kind="ExternalInput").ap()
    kt = nc.dram_tensor("kt", [NKT, 128, NG * 512], fp16, kind="ExternalInput").ap()
    vt = nc.dram_tensor("vt", [NKT, 128, 4 * D], fp16, kind="ExternalInput").ap()
    z = nc.dram_tensor("z", [4, 128, D], fp32, kind="ExternalOutput").ap()

    with tile.TileContext(nc) as tc:
        for rep in range(repeat):
            with ExitStack() as ctx:
                _body(nc, tc, ctx, bass, mybir, qt, kt, vt, z, f"r{rep}_")
    nc.compile()
    return nc


def _body(nc, tc, ctx, bass, mybir, qt, kt, vt, z, R):
    fp16 = mybir.dt.float16
    fp32 = mybir.dt.float32
    AX = mybir.AxisListType
    AF = mybir.ActivationFunctionType

    const = ctx.enter_context(tc.tile_pool(name=R + "const", bufs=1))
    ssb = ctx.enter_context(tc.tile_pool(name=R + "ssb", bufs=1))
    stats = ctx.enter_context(tc.tile_pool(name=R + "stats", bufs=1))

    qt_sb = const.tile([128, NG, QPT], fp16, name=R + "qt_sb")
    nc.sync.dma_start(qt_sb[:], qt[:])
    s_sb = [
        ssb.tile([128, NKT * 512], fp16, tag=f"s{s}", name=R + f"s_sb{s}")
        for s in range(4)
    ]
    nm_all = stats.tile([128, 4, NKT], fp32, name=R + "nm_all")  # -rowmax

    # ---- pass 1: scores -> centered fp16 stash + per-tile row maxes ----
    with tc.tile_pool(name=R + "ktp", bufs=3) as ktp, \
         tc.tile_pool(name=R + "ps1", bufs=4, space="PSUM") as ps1:
        for t in range(NKT):
            kt_t = ktp.tile([128, NG, 512], fp16, name=R + "kt_t")
            nc.sync.dma_start(kt_t[:], kt[t])
            for s in range(4):
                ps = ps1.tile([128, 512], fp32, name=R + "ps")
                for g in range(NG):
                    nc.tensor.matmul(
                        ps[:],
                        qt_sb[:, g, bass.ts(s, 128)],
                        kt_t[:, g, :],
                        start=(g == 0),
                        stop=(g == NG - 1),
                    )
                nc.vector.reduce_max(
                    nm_all[:, s, t : t + 1], ps[:], axis=AX.X, negate=True
                )
                nc.scalar.activation(
                    s_sb[s][:, bass.ts(t, 512)],
                    ps[:],
                    AF.Identity,
                    bias=nm_all[:, s, t : t + 1],
                    scale=1.0,
                )

    mn4 = stats.tile([128, 4], fp32, name=R + "mn4")  # min_t(-m_t) = -m_glob
    nc.vector.tensor_reduce(mn4[:], nm_all[:], axis=AX.X, op=mybir.AluOpType.min)
    df = stats.tile([128, 4, NKT], fp32, name=R + "df")  # m_glob - m_t >= 0
    for s in range(4):
        nc.vector.tensor_scalar_sub(df[:, s, :], nm_all[:, s, :], mn4[:, s : s + 1])
    b_all = stats.tile([128, 4, NKT], fp32, name=R + "b_all")  # 10*(m_t - m)
    nc.scalar.mul(b_all[:], df[:], -SCALE)

    l_all = stats.tile([128, 4, NKT], fp32, name=R + "l_all")

    # ---- pass 2: exp -> transpose -> P@V accumulate (512-kv groups) ----
    with tc.tile_pool(name=R + "vtp", bufs=4) as vtp, \
         tc.tile_pool(name=R + "pp", bufs=12) as pp, \
         tc.tile_pool(name=R + "ptp", bufs=12) as ptp, \
         tc.tile_pool(name=R + "acc", bufs=1, space="PSUM") as accp:
        acc = [
            accp.tile([128, D], fp32, tag=f"acc{s}", name=R + f"acc{s}")
            for s in range(4)
        ]
        for grp in range(NKT):
            vt_g = vtp.tile([128, 4, D], fp16, name=R + "vt_g")
            nc.sync.dma_start(vt_g[:], vt[grp])
            for s in range(4):
                p = pp.tile([128, 512], fp16, name=R + "p")
                nc.scalar.activation(
                    p[:],
                    s_sb[s][:, bass.ts(grp, 512)],
                    AF.Exp,
                    bias=b_all[:, s, grp : grp + 1],
                    scale=SCALE,
                    accum_out=l_all[:, s, grp : grp + 1],
                )
                pt = ptp.tile([128, 4, 128], fp16, name=R + "pt")
                nc.sync.dma_start_transpose(pt[:], p[:])
                for i in range(4):
                    first = grp == 0 and i == 0
                    nc.tensor.matmul(
                        acc[s][:, 0:512], pt[:, i, :], vt_g[:, i, 0:512],
                        start=first, stop=False,
                    )
                    nc.tensor.matmul(
                        acc[s][:, 512:D], pt[:, i, :], vt_g[:, i, 512:D],
                        start=first, stop=(grp == NKT - 1 and i == 3),
                    )

        # ---- finalize: Z = acc / rowsum ----
        with tc.tile_pool(name=R + "zp", bufs=2) as zp:
            for s in range(4):
                l = stats.tile([128, 1], fp32, tag=f"l{s}", name=R + f"l{s}")
                nc.vector.reduce_sum(l[:], l_all[:, s, :], axis=AX.X)
                r = stats.tile([128, 1], fp32, tag=f"r{s}", name=R + f"r{s}")
                nc.vector.reciprocal(r[:], l[:])
                zt = zp.tile([128, D], fp32, name=R + "zt")
                nc.vector.tensor_scalar_mul(zt[:], acc[s][:], r[:])
                nc.sync.dma_start(z[s], zt[:])


def _unfold_om(xp, stride, n):
    """xp [CIT, Hp, Wp] -> [49, CIT, n*n], offset-major (o = i*7+j)."""
    s = xp.strides
    w = as_strided(
        xp,
        shape=(KS, KS, CIT, n, n),
        strides=(s[1], s[2], s[0], s[1] * stride, s[2] * stride),
    )
    return w.reshape(KS * KS, CIT, n * n)


def _pad_group(u, ncols):
    """[49, CIT, ncols] -> [128, NG, ncols] with zero pad to 56 offsets."""
    up = np.zeros((NG * 8, CIT, ncols), u.dtype)
    up[: KS * KS] = u
    return (
        up.reshape(NG, 8, CIT, ncols)
        .transpose(1, 2, 0, 3)
        .reshape(128, NG, ncols)
    )


def _prep_inputs(b, g_w, g_b, th_w, th_b, ph_w, ph_b):
    bf = b.reshape(B, CIN, H * W)
    mk = lambda w, bi: (
        np.einsum("oc,bcp->bop", np.asarray(w, np.float32), bf)
        + np.asarray(bi, np.float32)[None, :, None]
    ).reshape(B, CIT, H, W)
    b1, b2, b3 = mk(g_w, g_b), mk(th_w, th_b), mk(ph_w, ph_b)

    pad3 = lambda x: np.pad(x, ((0, 0), (PAD, PAD), (PAD, PAD)))
    in_maps = []
    for img in range(B):
        b1p, b2p, b3p = pad3(b1[img]), pad3(b2[img]), pad3(b3[img])

        u1 = _unfold_om(b1p, SQ, NQ)                          # [49, 16, 1024]
        qt_full = _pad_group(u1, NQ * NQ).astype(np.float16)  # [128, 7, 1024]

        u3 = _unfold_om(b3p, 1, 128)                          # [49, 16, 16384]
        ktg = _pad_group(u3, L2).astype(np.float16)           # [128, 7, 16384]
        kt = np.ascontiguousarray(
            ktg.reshape(128, NG, NKT, 512).transpose(2, 0, 1, 3)
        ).reshape(NKT, 128, NG * 512)

        s = b2p.strides
        wv = as_strided(
            b2p,
            shape=(CIT, 128, 128, KS, KS),
            strides=(s[0], s[1], s[2], s[1], s[2]),
        )
        vtn = (
            wv.transpose(1, 2, 3, 4, 0)
            .reshape(NKT, 4, 128, D)
            .transpose(0, 2, 1, 3)
            .reshape(NKT, 128, 4 * D)
            .astype(np.float16)
        )
        vtn = np.ascontiguousarray(vtn)

        for half in range(2):
            in_maps.append(
                {
                    "qt": np.ascontiguousarray(
                        qt_full[:, :, half * QPT : (half + 1) * QPT]
                    ),
                    "kt": kt,
                    "vt": vtn,
                }
            )
    return in_maps


def _postprocess(results, b, o_w, o_b):
    cnt1 = np.zeros(H + 2 * PAD, np.float32)
    for i in range(KS):
        cnt1[i : i + (NQ - 1) * SQ + 1 : SQ] += 1.0
    cnt = np.outer(cnt1, cnt1)[PAD : PAD + H, PAD : PAD + W]

    out = np.empty((B, CIN, H, W), np.float32)
    o_w32 = np.asarray(o_w, np.float32)
    o_b32 = np.asarray(o_b, np.float32)
    for img in range(B):
        zi = np.concatenate(
            [results[img * 2 + h]["z"].reshape(QPT, D) for h in range(2)], 0
        )                                                     # [1024, 784]
        zr = zi.reshape(NQ, NQ, KS, KS, CIT)                  # qy qx i j c
        ypad = np.zeros((CIT, H + 2 * PAD, W + 2 * PAD), np.float32)
        for i in range(KS):
            for j in range(KS):
                ypad[
                    :, i : i + (NQ - 1) * SQ + 1 : SQ, j : j + (NQ - 1) * SQ + 1 : SQ
                ] += zr[:, :, i, j, :].transpose(2, 0, 1)
        y = ypad[:, PAD : PAD + H, PAD : PAD + W] / cnt[None]
        out[img] = (
            np.einsum("oc,chw->ohw", o_w32, y)
            + o_b32[:, None, None]
            + b[img]
        )
    return out


def kernel(b, g_w, g_b, th_w, th_b, ph_w, ph_b, o_w, o_b):
    from concourse.bass_utils import run_bass_kernel_spmd
    import os
    import time as _time

    b = np.asarray(b, np.float32)
    if "nc" not in _cache:
        _cache["nc"] = _build_bass()
    nc = _cache["nc"]

    in_maps = _prep_inputs(b, g_w, g_b, th_w, th_b, ph_w, ph_b)

    trace = bool(int(os.environ.get("KERNEL_TRACE", "0")))
    t0 = _time.time()
    res = run_bass_kernel_spmd(nc, in_maps, core_ids=list(range(8)), trace=trace)
    _cache["exec_wall_s"] = _time.time() - t0
    _cache["last_results"] = res

    return _postprocess(res.results, b, o_w, o_b)


# revision 18
# speedup vs baseline: 543.3352x; 543.3352x over previous
"""Contextual patch attention (B=4, Cin=64, H=W=128) on 8 trn2 NeuronCores.

Sharding: core = img*2 + half. Each core handles one image's 512 query
patches (half of 1024) against all 16384 key/value patches of that image.
Host (numpy) does the cheap 1x1 convs + unfold layout prep and the final
fold / output conv / residual; the device does the ~210 GFLOP attention.
"""

import numpy as np
from contextlib import ExitStack
from numpy.lib.stride_tricks import as_strided

KS = 7
PAD = 3
SQ = 4          # query stride
SCALE = 10.0
B, CIN, H, W = 4, 64, 128, 128
CIT = 16
NQ = 32         # query grid 32x32
L2 = 128 * 128
D = CIT * KS * KS          # 784
NG = 7                     # contraction groups of 128 (pad 49 offsets -> 56)
QPT = 512                  # queries per core
NKT = 32                   # kv tiles of 512

_cache = {}


def _build_bass(repeat=1):
    import concourse.bass as bass
    import concourse.mybir as mybir
    import concourse.tile as tile
    from concourse import bacc

    fp16 = mybir.dt.float16
    fp32 = mybir.dt.float32

    nc = bacc.Bacc("TRN2", target_bir_lowering=False, debug=False, num_devices=8)
    qt = nc.dram_tensor("qt", [128, NG, QPT], fp16, kind="ExternalInput").ap()
    kt = nc.dram_tensor("kt", [NKT, 128, NG * 512], fp16, kind="ExternalInput").ap()
    vt = nc.dram_tensor("vt", [NKT, 128, 4 * D], fp16, kind="ExternalInput").ap()
    z = nc.dram_tensor("z", [4, 128, D], fp32, kind="ExternalOutput").ap()

    with tile.TileContext(nc) as tc:
        for rep in range(repeat):
            with ExitStack() as ctx:
                _body(nc, tc, ctx, bass, mybir, qt, kt, vt, z, f"r{rep}_")
    nc.compile()
    return nc


def _body(nc, tc, ctx, bass, mybir, qt, kt, vt, z, R):
    fp16 = mybir.dt.float16
    fp32 = mybir.dt.float32
    AX = mybir.AxisListType
    AF = mybir.ActivationFunctionType

    const = ctx.enter_context(tc.tile_pool(name=R + "const", bufs=1))
    ssb = ctx.enter_context(tc.tile_pool(name=R + "ssb", bufs=1))
    stats = ctx.enter_context(tc.tile_pool(name=R + "stats", bufs=1))

    qt_sb = const.tile([128, NG, QPT], fp16, name=R + "qt_sb")
    nc.sync.dma_start(qt_sb[:], qt[:])
    s_sb = [
        ssb.tile([128, NKT * 512], fp16, tag=f"s{s}", name=R + f"s_sb{s}")
        for s in range(4)
    ]
    nm_all = stats.tile([128, 4, NKT], fp32, name=R + "nm_all")  # -rowmax

    # ---- pass 1: scores -> centered fp16 stash + per-tile row maxes ----
    with tc.tile_pool(name=R + "ktp", bufs=4) as ktp, \
         tc.tile_pool(name=R + "ps1", bufs=2, space="PSUM") as ps1:
        for t in range(NKT):
            kt_t = ktp.tile([128, NG, 512], fp16, name=R + "kt_t")
            nc.sync.dma_start(kt_t[:], kt[t])
            pss = []
            for s in range(4):
                ps = ps1.tile([128, 512], fp32, name=R + f"ps{s}", tag=f"ps{s}")
                pss.append(ps)
                for g in range(NG - 1):
                    nc.tensor.matmul(
                        ps[:],
                        qt_sb[:, g, bass.ts(s, 128)],
                        kt_t[:, g, :],
                        start=(g == 0),
                        stop=False,
                    )
            # tail group (K=16, offset 48): 4 subtiles packed into the four
            # 32-row PE groups -> they run concurrently on hardware
            for s in range(4):
                nc.tensor.matmul(
                    pss[s][:],
                    qt_sb[32 * s : 32 * s + 16, NG - 1, bass.ts(s, 128)],
                    kt_t[32 * s : 32 * s + 16, NG - 1, :],
                    start=False,
                    stop=True,
                    tile_position=(32 * s, 0),
                )
            for s in range(4):
                nc.vector.reduce_max(
                    nm_all[:, s, t : t + 1], pss[s][:], axis=AX.X, negate=True
                )
                nc.scalar.activation(
                    s_sb[s][:, bass.ts(t, 512)],
                    pss[s][:],
                    AF.Identity,
                    bias=nm_all[:, s, t : t + 1],
                    scale=1.0,
                )

    mn4 = stats.tile([128, 4], fp32, name=R + "mn4")  # min_t(-m_t) = -m_glob
    nc.vector.tensor_reduce(mn4[:], nm_all[:], axis=AX.X, op=mybir.AluOpType.min)
    df = stats.tile([128, 4, NKT], fp32, name=R + "df")  # m_glob - m_t >= 0
    for s in range(4):
        nc.vector.tensor_scalar_sub(df[:, s, :], nm_all[:, s, :], mn4[:, s : s + 1])
    b_all = stats.tile([128, 4, NKT], fp32, name=R + "b_all")  # 10*(m_t - m)
    nc.scalar.mul(b_all[:], df[:], -SCALE)

    l_all = stats.tile([128, 4, NKT], fp32, name=R + "l_all")

    # ---- pass 2: exp -> transpose -> P@V accumulate (512-kv groups) ----
    with tc.tile_pool(name=R + "vtp", bufs=4) as vtp, \
         tc.tile_pool(name=R + "pp", bufs=4) as pp, \
         tc.tile_pool(name=R + "ptp", bufs=4) as ptp, \
         tc.tile_pool(name=R + "acc", bufs=1, space="PSUM") as accp:
        acc = [
            accp.tile([128, D], fp32, tag=f"acc{s}", name=R + f"acc{s}")
            for s in range(4)
        ]
        for grp in range(NKT):
            vt_g = vtp.tile([128, 4, D], fp16, name=R + "vt_g")
            nc.sync.dma_start(vt_g[:], vt[grp])
            p_all = pp.tile([128, 4, 512], fp16, name=R + "p_all")
            for s in range(4):
                nc.scalar.activation(
                    p_all[:, s, :],
                    s_sb[s][:, bass.ts(grp, 512)],
                    AF.Exp,
                    bias=b_all[:, s, grp : grp + 1],
                    scale=SCALE,
                    accum_out=l_all[:, s, grp : grp + 1],
                )
            # one batched 32-block transpose per group: block 4s+i = P_s[:, i].T
            pt_all = ptp.tile([128, 16, 128], fp16, name=R + "pt_all")
            nc.scalar.dma_start_transpose(pt_all[:], p_all[:])
            for s in range(4):
                for i in range(4):
                    first = grp == 0 and i == 0
                    nc.tensor.matmul(
                        acc[s][:, 0:512], pt_all[:, 4 * s + i, :],
                        vt_g[:, i, 0:512],
                        start=first, stop=False,
                    )
                    nc.tensor.matmul(
                        acc[s][:, 512:D], pt_all[:, 4 * s + i, :],
                        vt_g[:, i, 512:D],
                        start=first, stop=(grp == NKT - 1 and i == 3),
                    )

        # ---- finalize: Z = acc / rowsum ----
        with tc.tile_pool(name=R + "zp", bufs=2) as zp:
            for s in range(4):
                l = stats.tile([128, 1], fp32, tag=f"l{s}", name=R + f"l{s}")
                nc.vector.reduce_sum(l[:], l_all[:, s, :], axis=AX.X)
                r = stats.tile([128, 1], fp32, tag=f"r{s}", name=R + f"r{s}")
                nc.vector.reciprocal(r[:], l[:])
                zt = zp.tile([128, D], fp32, name=R + "zt")
                nc.vector.tensor_scalar_mul(zt[:], acc[s][:], r[:])
                nc.sync.dma_start(z[s], zt[:])


def _unfold_om(xp, stride, n):
    """xp [CIT, Hp, Wp] -> [49, CIT, n*n], offset-major (o = i*7+j)."""
    s = xp.strides
    w = as_strided(
        xp,
        shape=(KS, KS, CIT, n, n),
        strides=(s[1], s[2], s[0], s[1] * stride, s[2] * stride),
    )
    return w.reshape(KS * KS, CIT, n * n)


def _pad_group(u, ncols):
    """[49, CIT, ncols] -> [128, NG, ncols], offsets 0-47 in groups 0-5.

    Offset 48 is replicated at pseudo-offsets 50/52/54 so the last group's
    rows land at partitions {0,32,64,96}+c — one copy per PE 32-row group
    for the row-packed tail matmuls.
    """
    up = np.zeros((NG * 8, CIT, ncols), u.dtype)
    up[: KS * KS] = u
    for k in (1, 2, 3):
        up[48 + 2 * k] = u[48]
    return (
        up.reshape(NG, 8, CIT, ncols)
        .transpose(1, 2, 0, 3)
        .reshape(128, NG, ncols)
    )


def _prep_inputs(b, g_w, g_b, th_w, th_b, ph_w, ph_b):
    bf = b.reshape(B, CIN, H * W)
    mk = lambda w, bi: (
        np.einsum("oc,bcp->bop", np.asarray(w, np.float32), bf)
        + np.asarray(bi, np.float32)[None, :, None]
    ).reshape(B, CIT, H, W)
    b1, b2, b3 = mk(g_w, g_b), mk(th_w, th_b), mk(ph_w, ph_b)

    pad3 = lambda x: np.pad(x, ((0, 0), (PAD, PAD), (PAD, PAD)))
    in_maps = []
    for img in range(B):
        b1p, b2p, b3p = pad3(b1[img]), pad3(b2[img]), pad3(b3[img])

        u1 = _unfold_om(b1p, SQ, NQ)                          # [49, 16, 1024]
        qt_full = _pad_group(u1, NQ * NQ).astype(np.float16)  # [128, 7, 1024]

        u3 = _unfold_om(b3p, 1, 128)                          # [49, 16, 16384]
        ktg = _pad_group(u3, L2).astype(np.float16)           # [128, 7, 16384]
        kt = np.ascontiguousarray(
            ktg.reshape(128, NG, NKT, 512).transpose(2, 0, 1, 3)
        ).reshape(NKT, 128, NG * 512)

        s = b2p.strides
        wv = as_strided(
            b2p,
            shape=(CIT, 128, 128, KS, KS),
            strides=(s[0], s[1], s[2], s[1], s[2]),
        )
        vtn = (
            wv.transpose(1, 2, 3, 4, 0)
            .reshape(NKT, 4, 128, D)
            .transpose(0, 2, 1, 3)
            .reshape(NKT, 128, 4 * D)
            .astype(np.float16)
        )
        vtn = np.ascontiguousarray(vtn)

        for half in range(2):
            in_maps.append(
                {
                    "qt": np.ascontiguousarray(
                        qt_full[:, :, half * QPT : (half + 1) * QPT]
                    ),
                    "kt": kt,
                    "vt": vtn,
                }
            )
    return in_maps


def _postprocess(results, b, o_w, o_b):
    cnt1 = np.zeros(H + 2 * PAD, np.float32)
    for i in range(KS):
        cnt1[i : i + (NQ - 1) * SQ + 1 : SQ] += 1.0
    cnt = np.outer(cnt1, cnt1)[PAD : PAD + H, PAD : PAD + W]

    out = np.empty((B, CIN, H, W), np.float32)
    o_w32 = np.asarray(o_w, np.float32)
    o_b32 = np.asarray(o_b, np.float32)
    for img in range(B):
        zi = np.concatenate(
            [results[img * 2 + h]["z"].reshape(QPT, D) for h in range(2)], 0
        )                                                     # [1024, 784]
        zr = zi.reshape(NQ, NQ, KS, KS, CIT)                  # qy qx i j c
        ypad = np.zeros((CIT, H + 2 * PAD, W + 2 * PAD), np.float32)
        for i in range(KS):
            for j in range(KS):
                ypad[
                    :, i : i + (NQ - 1) * SQ + 1 : SQ, j : j + (NQ - 1) * SQ + 1 : SQ
                ] += zr[:, :, i, j, :].transpose(2, 0, 1)
        y = ypad[:, PAD : PAD + H, PAD : PAD + W] / cnt[None]
        out[img] = (
            np.einsum("oc,chw->ohw", o_w32, y)
            + o_b32[:, None, None]
            + b[img]
        )
    return out


def kernel(b, g_w, g_b, th_w, th_b, ph_w, ph_b, o_w, o_b):
    from concourse.bass_utils import run_bass_kernel_spmd
    import os
    import time as _time

    b = np.asarray(b, np.float32)
    if "nc" not in _cache:
        _cache["nc"] = _build_bass()
    nc = _cache["nc"]

    in_maps = _prep_inputs(b, g_w, g_b, th_w, th_b, ph_w, ph_b)

    trace = bool(int(os.environ.get("KERNEL_TRACE", "0")))
    t0 = _time.time()
    res = run_bass_kernel_spmd(nc, in_maps, core_ids=list(range(8)), trace=trace)
    _cache["exec_wall_s"] = _time.time() - t0
    _cache["last_results"] = res

    return _postprocess(res.results, b, o_w, o_b)
